# revision 39
# baseline (speedup 1.0000x reference)
"""Trainium2 Bass kernel for a GPT-2-style transformer block.

B=1, T=4096, C=768, H=12 heads (hd=64), causal attention, exact GELU MLP.

Distribution over 8 NeuronCores (single shared SPMD program; collectives on
this pool measure ~0.4-1 ms per call, so the design avoids them entirely):
  - Queries: mod-8 interleaved sharding (core c owns tokens t with t%8==c),
    which makes the causal-attention instruction structure IDENTICAL on all
    cores (one shared program; per-core behavior only via input data). The
    per-core diagonal-band causal masks are fed as inputs.
  - K/V: every core computes the full-sequence K^T/V locally (replicated
    matmul — far cheaper than any collective here). K and V projections run
    in fp8e4m3 with DoubleRow packing (2x PE rate) off a shared fp8 copy of
    the LN output (xp8).
  - K^T and Q^T are stored fp8, DoubleRow-packed along hd (head h on
    partitions 32*(h%3):+32 — matmul operands must base at 0/32/64 — group
    h//3, hd = 32*j + p with j a free dim), so the S matmuls also run at
    the 2x fp8 rate. The pack is 4 small SBUF DMAs per produced psum tile;
    the Q/K weight columns are host-permuted so psum partition 64j+32e+q
    holds pair-local feature 64e+32j+q, making each (head, j) chunk a
    contiguous 32-partition block.
  - V is SBUF-resident fp8 in natural [token, feature] layout with a
    prepended ones-column per head ([P, 32, 12, 65]): the AV matmul then
    accumulates the softmax row-sum into output partition 0 for free (no
    separate row-sum matmuls). Per-pair normalization: DVE reciprocal of
    the two row-sum rows, GPSIMD partition_broadcast, two muls.
  - Slab production (LN1 + K/V for token slabs 2..7) is INTERLEAVED into
    pair 0's attention at half-band granularity (band m half h needs
    exactly slab 2m+h), so the DVE/PE-heavy K/V work overlaps the
    Act-bound softmax exp of pair 0; pairs 1-5 then run Act-bound.
  - proj/LN2/MLP/residual: row-parallel on each core's own query rows.
    The MLP stays bf16: its output is ~30% of the residual, fp8 there
    costs ~1.6e-2 rel err (vs the ~1% attention branch where fp8 K/Q/V
    error is diluted ~100x).

LN statistics use ones-matmul partition reductions into a [P, 2, R] psum
tile SHARED (same pool tag) with the attention score tiles, keeping the
PSUM budget at 8 banks: scores/stats (2) + qk (2) + v (2) + y0/y1 (2).
rstd = Sqrt(reciprocal(var+eps)) (Rsqrt activation is banned; the Ln/Exp
trick thrashes the activation table at 1.28us per load since Ln and Exp
resolve to different tables). LN1 for the K/V slabs writes fp8 directly.
"""

import numpy as np
import ml_dtypes

import concourse.bacc as bacc
import concourse.mybir as mybir
import concourse.tile as tile
from concourse.bass_utils import run_bass_kernel_spmd

BF16 = ml_dtypes.bfloat16
F8 = ml_dtypes.float8_e4m3

# problem shape (hardcoded per harness contract)
T = 4096
C = 768
H = 12
HD = 64
EPS = 1e-5
NC = 8          # cores
R = 512         # tokens per core
P = 128
CT = C // P     # 6 feature tiles
QT = R // P     # 4 query tiles per core
KT = T // P     # 32 key tiles
PAIRS = H // 2  # 6 head pairs
HT = (4 * C) // P  # 24 hidden tiles

_CACHE = {}


def _ln_transposed(nc, tc, pool, pspool, xT, out_bf, ones_sb, w_col, b_col, apply_wb,
                   x_is_bf16=False, alt=0):
    """LayerNorm over the feature axis for [C, R]-transposed activations.

    xT: f32 (or bf16 with x_is_bf16) sbuf tile [P, CT, R]; out_bf: bf16 or
    fp8 tile. Stats via ones-matmul partition reduction (all-partition-
    broadcast results) into one [P, 2, R] psum tile (plane 0 = sum,
    plane 1 = sumsq); rstd = Sqrt(reciprocal(var+eps)).
    """
    f32 = mybir.dt.float32
    bf16 = mybir.dt.bfloat16
    if x_is_bf16:
        xb = xT
    else:
        xb = pool.tile([P, CT, R], bf16, tag="ln_xb")
    sq = pool.tile([P, CT, R], bf16, tag="ln_sq")
    for k in range(CT):
        eng = nc.gpsimd if (k + alt) % 3 == 2 else nc.vector
        if not x_is_bf16:
            eng.tensor_copy(xb[:, k, :], xT[:, k, :])
        eng.tensor_mul(sq[:, k, :], xb[:, k, :], xb[:, k, :])
    st = pspool.tile([P, 2, R], f32, tag="sl")
    for k in range(CT):
        nc.tensor.matmul(st[:, 0, :], ones_sb[:], xb[:, k, :], start=(k == 0), stop=(k == CT - 1))
    for k in range(CT):
        nc.tensor.matmul(st[:, 1, :], ones_sb[:], sq[:, k, :], start=(k == 0), stop=(k == CT - 1))
    # scratch tiles are reused in place: A = nmean -> -mu*rstd,
    # B = E[x^2] -> var -> rstd, tmp doubles as mean^2
    nmean = pool.tile([P, R], f32, tag="ln_a")
    rstd = pool.tile([P, R], f32, tag="ln_b")
    tmp = pool.tile([P, R], f32, tag="ln_tmp")
    tmp2 = pool.tile([P, R], f32, tag="ln_tmp2")
    sm = nc.vector if alt == 0 else nc.gpsimd
    sm.tensor_scalar_mul(nmean[:], st[:, 0, :], -1.0 / C)
    sm.tensor_scalar_mul(rstd[:], st[:, 1, :], 1.0 / C)   # E[x^2]
    sm.tensor_mul(tmp[:], nmean[:], nmean[:])             # mean^2
    # var = (E[x^2] + eps) - mean^2
    sm.scalar_tensor_tensor(
        rstd[:], rstd[:], EPS, tmp[:], mybir.AluOpType.add, mybir.AluOpType.subtract
    )
    nc.vector.reciprocal(rstd[:], rstd[:])
    nc.scalar.activation(rstd[:], rstd[:], mybir.ActivationFunctionType.Sqrt)
    nmr = nmean
    sm.tensor_mul(nmr[:], nmean[:], rstd[:])           # -mu*rstd
    for k in range(CT):
        eng = nc.gpsimd if (k + alt) % 3 == 2 else nc.vector
        t = tmp2 if (k + alt) % 3 == 2 else tmp
        eng.tensor_mul(t[:], xT[:, k, :], rstd[:])
        if apply_wb:
            eng.tensor_add(t[:], t[:], nmr[:])
            eng.tensor_scalar(
                out_bf[:, k, :], t[:], w_col[:, k : k + 1], b_col[:, k : k + 1],
                mybir.AluOpType.mult, mybir.AluOpType.add,
            )
        else:
            eng.tensor_add(out_bf[:, k, :], t[:], nmr[:])


def _build(apply_ln1, apply_ln2, apply_bv, apply_bqk=False, sim_no_cc=False, reps=1):
    key = (apply_ln1, apply_ln2, apply_bv, apply_bqk, reps)
    if key in _CACHE:
        return _CACHE[key]

    f32 = mybir.dt.float32
    bf16 = mybir.dt.bfloat16
    f8 = mybir.dt.float8e4
    AF = mybir.ActivationFunctionType
    DR = mybir.MatmulPerfMode.DoubleRow

    nc = bacc.Bacc("TRN2", target_bir_lowering=False, debug=False, num_devices=NC)

    xtf_d = nc.declare_dram_parameter("xtf", [C, T], bf16, isOutput=False)
    xTq_d = nc.declare_dram_parameter("xTq", [C, R], f32, isOutput=False)
    masks_d = nc.declare_dram_parameter("masks", [P, 8, P], bf16, isOutput=False)
    ones_d = nc.declare_dram_parameter("ones", [P, P], bf16, isOutput=False)
    wq8_d = nc.declare_dram_parameter("wq8", [HD, CT, 2, C], f8, isOutput=False)
    wk8_d = nc.declare_dram_parameter("wk8", [HD, CT, 2, C], f8, isOutput=False)
    wv8_d = nc.declare_dram_parameter("wv8", [HD, CT, 2, C], f8, isOutput=False)
    wproj_d = nc.declare_dram_parameter("wproj", [C, C], bf16, isOutput=False)
    wfc_d = nc.declare_dram_parameter("wfc", [C, 4 * C], bf16, isOutput=False)
    wfc2_d = nc.declare_dram_parameter("wfc2", [4 * C, C], bf16, isOutput=False)
    bqk_d = nc.declare_dram_parameter("bqk", [P, 2 * CT], f32, isOutput=False)
    bproj_d = nc.declare_dram_parameter("bproj", [P, CT], f32, isOutput=False)
    bfc_d = nc.declare_dram_parameter("bfc", [P, HT], f32, isOutput=False)
    bfc2_d = nc.declare_dram_parameter("bfc2", [P, CT], f32, isOutput=False)
    if apply_bv:
        bv_d = nc.declare_dram_parameter("bv", [P, C], f32, isOutput=False)
    if apply_ln1:
        ln1w_d = nc.declare_dram_parameter("ln1w", [P, CT], f32, isOutput=False)
        ln1b_d = nc.declare_dram_parameter("ln1b", [P, CT], f32, isOutput=False)
    if apply_ln2:
        ln2w_d = nc.declare_dram_parameter("ln2w", [P, CT], f32, isOutput=False)
        ln2b_d = nc.declare_dram_parameter("ln2b", [P, CT], f32, isOutput=False)
    outT_d = nc.declare_dram_parameter("outT", [C, R], f32, isOutput=True)

    with tile.TileContext(nc) as tc:
        with (
            tc.tile_pool(name="const", bufs=1) as const,
            tc.tile_pool(name="mid", bufs=1) as mid,
        ):
            ones_sb = const.tile([P, P], bf16)
            nc.sync.dma_start(ones_sb[:], ones_d[:])
            bqk_sb = const.tile([P, 2 * CT], f32)
            nc.sync.dma_start(bqk_sb[:], bqk_d[:])
            bproj_sb = const.tile([P, CT], f32)
            nc.gpsimd.dma_start(bproj_sb[:], bproj_d[:])
            bfc_sb = const.tile([P, HT], f32)
            nc.gpsimd.dma_start(bfc_sb[:], bfc_d[:])
            bfc2_sb = const.tile([P, CT], f32)
            nc.gpsimd.dma_start(bfc2_sb[:], bfc2_d[:])
            if apply_bv:
                bv_sb = const.tile([P, C], f32)
                nc.sync.dma_start(bv_sb[:], bv_d[:])
            ln1w_sb = ln1b_sb = ln2w_sb = ln2b_sb = None
            if apply_ln1:
                ln1w_sb = const.tile([P, CT], f32)
                ln1b_sb = const.tile([P, CT], f32)
                nc.sync.dma_start(ln1w_sb[:], ln1w_d[:])
                nc.sync.dma_start(ln1b_sb[:], ln1b_d[:])
            if apply_ln2:
                ln2w_sb = const.tile([P, CT], f32)
                ln2b_sb = const.tile([P, CT], f32)
                nc.sync.dma_start(ln2w_sb[:], ln2w_d[:])
                nc.sync.dma_start(ln2b_sb[:], ln2b_d[:])
            xTq_sb = const.tile([P, CT, R], f32)
            nc.sync.dma_start(xTq_sb[:], xTq_d.rearrange("(o p) t -> p o t", p=P))

            # mid-lifetime tiles
            q8 = mid.tile([P, H // 3, 2, R], f8)
            ynorm_sb = mid.tile([P, CT, R], bf16)  # normalized attn out (y^T)

            for _rep in range(reps):
                # pools/psum live across the fused produce+attend phase:
                # scores/LN-stats share one pool tag (2 banks) + qk(2) +
                # v(2) + y0/y1(2) = 8 banks.
                ktp_cm = tc.tile_pool(name="ktp", bufs=1)
                ktp = ktp_cm.__enter__()
                kt8 = ktp.tile([P, H // 3, 2, T], f8)
                v_res = ktp.tile([P, KT, H, 65], f8)
                masks_sb = ktp.tile([P, 8, P], bf16)
                nc.gpsimd.dma_start(masks_sb[:], masks_d[:])
                bp_cm = tc.tile_pool(name="bp", bufs=2)
                bp = bp_cm.__enter__()
                bn_cm = tc.tile_pool(name="bn", bufs=1)
                bn = bn_cm.__enter__()
                ps_sl_cm = tc.tile_pool(name="ps_sl", bufs=1, space="PSUM")
                ps_sl = ps_sl_cm.__enter__()
                ps_qk_cm = tc.tile_pool(name="ps_qk", bufs=2, space="PSUM")
                ps_qk = ps_qk_cm.__enter__()
                ps_v_cm = tc.tile_pool(name="ps_v", bufs=2, space="PSUM")
                ps_v = ps_v_cm.__enter__()
                ps_y_cm = tc.tile_pool(name="ps_y", bufs=1, space="PSUM")
                ps_y = ps_y_cm.__enter__()
                lnp_cm = tc.tile_pool(name="lnp", bufs=1)
                lnp = lnp_cm.__enter__()
                slabp_cm = tc.tile_pool(name="slabp", bufs=2)
                slabp = slabp_cm.__enter__()

                # ones column of v_res (read by every AV matmul)
                nc.gpsimd.tensor_copy(
                    v_res[:, :, :, 0:1],
                    ones_sb[:, 0:1].to_broadcast((P, KT, H, 1)),
                )

                # ---- setup: Q^T (fp8 DoubleRow-packed) ----
                with tc.tile_pool(name="qpool", bufs=1) as qpool:
                    wq8_sb = qpool.tile([HD, CT, 2, C], f8)
                    nc.sync.dma_start(wq8_sb[:], wq8_d[:])
                    xlnq8 = qpool.tile([P, CT, R], f8)
                    _ln_transposed(nc, tc, lnp, ps_sl, xTq_sb, xlnq8, ones_sb,
                                   ln1w_sb, ln1b_sb, apply_ln1)
                    xqp8 = qpool.tile([HD, CT, 2, R], f8)
                    nc.scalar.dma_start(xqp8[:, :, 0, :], xlnq8[0:HD, :, :])
                    nc.scalar.dma_start(xqp8[:, :, 1, :], xlnq8[HD:P, :, :])
                    for f in range(CT):
                        ps = ps_qk.tile([P, R], f32, tag="qk_ps")
                        for k in range(CT):
                            nc.tensor.matmul(
                                ps[:], wq8_sb[:, k, :, P * f : P * (f + 1)],
                                xqp8[:, k, :, :], start=(k == 0), stop=(k == CT - 1),
                                perf_mode=DR,
                            )
                        qst = qpool.tile([P, R], f8, tag="qst", name="qst", bufs=1)
                        nc.vector.tensor_scalar(
                            qst[:], ps[:], bqk_sb[:, f : f + 1], None,
                            mybir.AluOpType.add,
                        )
                        for e in range(2):
                            h = 2 * f + e
                            for j in range(2):
                                nc.scalar.dma_start(
                                    q8[32 * (h % 3) : 32 * (h % 3) + 32, h // 3, j, :],
                                    qst[64 * j + 32 * e : 64 * j + 32 * e + 32, :],
                                )

                    # slab tiles (shared with the interleaved phase below)
                    wk8_sb = slabp.tile([HD, CT, 2, C], f8, bufs=1)
                    nc.gpsimd.dma_start(wk8_sb[:], wk8_d[:])
                    wv8_sb = slabp.tile([HD, CT, 2, C], f8, bufs=1)
                    nc.gpsimd.dma_start(wv8_sb[:], wv8_d[:])
                    xtf_r = xtf_d.rearrange("(o p) t -> p o t", p=P)

                    def emit_slab(s):
                        """LN1 + K^T + V for token slab s (fp8 DoubleRow)."""
                        xv = slabp.tile([P, CT, R], bf16, tag="xv", name="xv")
                        xeng = nc.sync if s % 2 == 0 else nc.gpsimd
                        xeng.dma_start(xv[:], xtf_r[:, :, R * s : R * (s + 1)])
                        xln8 = slabp.tile([P, CT, R], f8, tag="xln8", name="xln8")
                        _ln_transposed(nc, tc, lnp, ps_sl, xv, xln8, ones_sb,
                                       ln1w_sb, ln1b_sb, apply_ln1, x_is_bf16=True)
                        xp8 = slabp.tile([HD, CT, 2, R], f8, tag="xp8", name="xp8")
                        nc.scalar.dma_start(xp8[:, :, 0, :], xln8[0:HD, :, :])
                        nc.scalar.dma_start(xp8[:, :, 1, :], xln8[HD:P, :, :])

                        for f in range(CT):
                            ps = ps_qk.tile([P, R], f32, tag="qk_ps")
                            for k in range(CT):
                                nc.tensor.matmul(
                                    ps[:], wk8_sb[:, k, :, P * f : P * (f + 1)],
                                    xp8[:, k, :, :], start=(k == 0), stop=(k == CT - 1),
                                    perf_mode=DR,
                                )
                            kst = slabp.tile([P, R], f8, tag="kst", name="kst")
                            keng = nc.vector if (f + s) % 2 == 0 else nc.gpsimd
                            if apply_bqk:
                                keng.tensor_scalar(
                                    kst[:], ps[:],
                                    bqk_sb[:, CT + f : CT + f + 1], None,
                                    mybir.AluOpType.add,
                                )
                            else:
                                keng.tensor_copy(kst[:], ps[:])
                            rot = [nc.sync, nc.scalar, nc.gpsimd, nc.sync]
                            for e in range(2):
                                h = 2 * f + e
                                for j in range(2):
                                    rot[(2 * e + j + f) % 4].dma_start(
                                        kt8[32 * (h % 3) : 32 * (h % 3) + 32, h // 3, j,
                                            R * s : R * (s + 1)],
                                        kst[64 * j + 32 * e : 64 * j + 32 * e + 32, :],
                                    )

                        for t in range(QT):
                            kt = QT * s + t
                            for hh in range(2):
                                ps = ps_v.tile([P, 384], f32, tag="v_ps")
                                for k in range(CT):
                                    nc.tensor.matmul(
                                        ps[:], xp8[:, k, :, P * t : P * (t + 1)],
                                        wv8_sb[:, k, :, 384 * hh : 384 * (hh + 1)],
                                        start=(k == 0), stop=(k == CT - 1),
                                        perf_mode=DR,
                                    )
                                dst = v_res[:, kt, 6 * hh : 6 * (hh + 1), 1:65]
                                psv = ps[:].rearrange("p (h f) -> p h f", h=6)
                                veng = nc.vector if (t + hh) % 2 == 0 else nc.gpsimd
                                if apply_bv:
                                    bvv = bv_sb[:, 384 * hh : 384 * (hh + 1)].rearrange(
                                        "p (h f) -> p h f", h=6
                                    )
                                    veng.tensor_add(dst, psv, bvv)
                                else:
                                    veng.tensor_copy(dst, psv)

                    emit_slab(0)
                    emit_slab(1)

                    # ---- attention; slabs 2-7 interleave into pair 0 ----
                    for pr in range(PAIRS):
                        y0_ps = ps_y.tile([P, R], f32, tag="y0")
                        y1_ps = ps_y.tile([P, R], f32, tag="y1")
                        for m in range(4):  # bands of 8 key-tiles
                            N = P * (4 - m)
                            # p_band is h-major: [P, head, ktile-in-band, R]
                            p_band = bp.tile([P, 2, 8, R], bf16, tag="p")
                            G = (1, 1, 2, 4)[m]
                            for half in range(2):
                                # band m half h consumes exactly slab 2m+h
                                if pr == 0 and m > 0:
                                    emit_slab(2 * m + half)
                                ng = 4 // G  # exp groups per half
                                for g in range(half * ng, (half + 1) * ng):
                                    s_ps = ps_sl.tile([P, 2, R], f32, tag="sl", name="s_ps")
                                    for dg in range(G):
                                        d = g * G + dg
                                        k = 8 * m + d
                                        for e in range(2):
                                            h = 2 * pr + e
                                            hb = 32 * (h % 3)
                                            hg = h // 3
                                            nc.tensor.matmul(
                                                s_ps[:, e, dg * N : (dg + 1) * N],
                                                kt8[hb : hb + 32, hg, :, P * k : P * (k + 1)],
                                                q8[hb : hb + 32, hg, :, 0:N],
                                                perf_mode=DR,
                                                skip_group_check=True,
                                            )
                                    if G == 1:
                                        nc.scalar.activation(
                                            p_band[:, :, g, :N], s_ps[:, :, :N],
                                            AF.Exp, scale=0.125,
                                        )
                                    else:
                                        nc.scalar.activation(
                                            p_band[:, :, g * G : (g + 1) * G, :N],
                                            s_ps[:].rearrange("p h (a n) -> p h a n", n=N),
                                            AF.Exp, scale=0.125,
                                        )
                                for d in range(4 * half, 4 * half + 4):
                                    k = 8 * m + d
                                    # causal mask on the diagonal col-group
                                    meng = nc.gpsimd if pr == 0 else nc.vector
                                    meng.tensor_mul(
                                        p_band[:, :, d, N - P : N],
                                        p_band[:, :, d, N - P : N],
                                        masks_sb[:, d : d + 1, :].to_broadcast((P, 2, P)),
                                    )
                                    # AV + row-sum in one matmul per head (V
                                    # ones-column -> row-sum in partition 0)
                                    nc.tensor.matmul(
                                        y0_ps[0:65, 0:N], v_res[:, k, 2 * pr, :],
                                        p_band[:, 0, d, :N],
                                        start=(k == 0), stop=(k == KT - 1),
                                        skip_group_check=True,
                                    )
                                    nc.tensor.matmul(
                                        y1_ps[0:65, 0:N], v_res[:, k, 2 * pr + 1, :],
                                        p_band[:, 1, d, :N],
                                        start=(k == 0), stop=(k == KT - 1),
                                        skip_group_check=True,
                                    )
                        # normalization: recip the row-sum rows, partition-
                        # broadcast on GPSIMD, scale the AV rows
                        recip0 = bn.tile([1, R], bf16, tag="recip0")
                        recip1 = bn.tile([1, R], bf16, tag="recip1")
                        with nc.allow_low_precision(reason="softmax denom recip"):
                            nc.vector.reciprocal(recip0[:], y0_ps[0:1, :])
                            nc.vector.reciprocal(recip1[:], y1_ps[0:1, :])
                        bc0 = bn.tile([P, R], bf16, tag="bc0")
                        bc1 = bn.tile([P, R], bf16, tag="bc1")
                        nc.gpsimd.partition_broadcast(bc0[:], recip0[:])
                        nc.gpsimd.partition_broadcast(bc1[:], recip1[:])
                        nc.vector.tensor_mul(
                            ynorm_sb[0:HD, pr, :], y0_ps[1:65, :], bc0[0:HD, :]
                        )
                        nc.vector.tensor_mul(
                            ynorm_sb[HD:P, pr, :], y1_ps[1:65, :], bc1[HD:P, :]
                        )

                slabp_cm.__exit__(None, None, None)
                lnp_cm.__exit__(None, None, None)
                ps_y_cm.__exit__(None, None, None)
                ps_v_cm.__exit__(None, None, None)
                ps_qk_cm.__exit__(None, None, None)
                ps_sl_cm.__exit__(None, None, None)
                bn_cm.__exit__(None, None, None)
                bp_cm.__exit__(None, None, None)
                ktp_cm.__exit__(None, None, None)

                # ---------------- Phase C: proj + LN2 + MLP + out ----------------
                with (
                    tc.tile_pool(name="mlpp", bufs=1) as mlpp,
                    tc.tile_pool(name="lnp2", bufs=1) as lnp2,
                ):
                    # weight loads split across queues so no single 14us DMA
                    # gates the MLP
                    wproj_sb = mlpp.tile([P, CT, C], bf16)
                    wproj_r = wproj_d.rearrange("(o p) f -> p o f", p=P)
                    nc.sync.dma_start(wproj_sb[:, 0:3, :], wproj_r[:, 0:3, :])
                    nc.gpsimd.dma_start(wproj_sb[:, 3:6, :], wproj_r[:, 3:6, :])
                    wfc_sb = mlpp.tile([P, CT, 4 * C], bf16)
                    wfc_r = wfc_d.rearrange("(o p) f -> p o f", p=P)
                    dengs = [nc.sync, nc.gpsimd, nc.scalar, nc.gpsimd]
                    for ch in range(4):
                        dengs[ch % 4].dma_start(
                            wfc_sb[:, :, C * ch : C * (ch + 1)],
                            wfc_r[:, :, C * ch : C * (ch + 1)],
                        )
                    z_sb = mlpp.tile([P, CT, R], f32)      # residual stream x+attn
                    xln2_sb = mlpp.tile([P, CT, R], bf16)
                    wfc2_sb = mlpp.tile([P, HT, C], bf16)
                    wfc2_r = wfc2_d.rearrange("(o p) f -> p o f", p=P)
                    for ch in range(4):
                        dengs[ch % 4].dma_start(
                            wfc2_sb[:, CT * ch : CT * (ch + 1), :],
                            wfc2_r[:, CT * ch : CT * (ch + 1), :],
                        )
                    with (
                        tc.tile_pool(name="ps_proj", bufs=2, space="PSUM") as ps_proj,
                        tc.tile_pool(name="ps_ln2", bufs=1, space="PSUM") as ps_ln2,
                    ):
                        for f in range(CT):
                            ps = ps_proj.tile([P, R], f32, tag="proj")
                            for k in range(CT):
                                nc.tensor.matmul(
                                    ps[:], wproj_sb[:, k, P * f : P * (f + 1)],
                                    ynorm_sb[:, k, :], start=(k == 0), stop=(k == CT - 1),
                                )
                            # z = (proj + b_proj) + x
                            nc.vector.scalar_tensor_tensor(
                                z_sb[:, f, :], ps[:], bproj_sb[:, f : f + 1], xTq_sb[:, f, :],
                                mybir.AluOpType.add, mybir.AluOpType.add,
                            )
                        _ln_transposed(nc, tc, lnp2, ps_ln2, z_sb, xln2_sb, ones_sb,
                                       ln2w_sb, ln2b_sb, apply_ln2)

                    h_sb = mlpp.tile([P, CT, R], bf16)
                    with (
                        tc.tile_pool(name="ps_fc1", bufs=2, space="PSUM") as ps_fc1,
                        tc.tile_pool(name="ps_o", bufs=1, space="PSUM") as ps_o,
                    ):
                        o_ps = [ps_o.tile([P, R], f32, tag=f"o{f}", name=f"o_ps{f}") for f in range(CT)]
                        for chunk in range(4):
                            for hf in range(CT):
                                hh = CT * chunk + hf
                                ps = ps_fc1.tile([P, R], f32, tag="fc1")
                                for k in range(CT):
                                    nc.tensor.matmul(
                                        ps[:], wfc_sb[:, k, P * hh : P * (hh + 1)],
                                        xln2_sb[:, k, :], start=(k == 0), stop=(k == CT - 1),
                                    )
                                nc.scalar.activation(
                                    h_sb[:, hf, :], ps[:], AF.Gelu, bias=bfc_sb[:, hh : hh + 1]
                                )
                                for f in range(CT):
                                    nc.tensor.matmul(
                                        o_ps[f][:], wfc2_sb[:, hh, P * f : P * (f + 1)],
                                        h_sb[:, hf, :], start=(hh == 0), stop=(hh == HT - 1),
                                    )
                        for f in range(CT):
                            # out = (o + b_fc2) + z, in place over z
                            nc.vector.scalar_tensor_tensor(
                                z_sb[:, f, :], o_ps[f][:], bfc2_sb[:, f : f + 1], z_sb[:, f, :],
                                mybir.AluOpType.add, mybir.AluOpType.add,
                            )
                            nc.sync.dma_start(outT_d[P * f : P * (f + 1), :], z_sb[:, f, :])

    nc.compile()
    _CACHE[key] = nc
    return nc


def _query_tokens(c):
    """Token ids owned by core c, in on-chip column order (j desc, i asc)."""
    return np.concatenate([1024 * j + 8 * np.arange(P) + c for j in (3, 2, 1, 0)])


def kernel(x, ln1_w, ln1_b, W_attn, b_attn, W_proj, b_proj,
           ln2_w, ln2_b, W_fc, b_fc, W_fc2, b_fc2):
    x = np.asarray(x, np.float32)
    ln1_w = np.asarray(ln1_w, np.float32)
    ln1_b = np.asarray(ln1_b, np.float32)
    W_attn = np.asarray(W_attn, np.float32)
    b_attn = np.asarray(b_attn, np.float32)
    W_proj = np.asarray(W_proj, np.float32)
    b_proj = np.asarray(b_proj, np.float32)
    ln2_w = np.asarray(ln2_w, np.float32)
    ln2_b = np.asarray(ln2_b, np.float32)
    W_fc = np.asarray(W_fc, np.float32)
    b_fc = np.asarray(b_fc, np.float32)
    W_fc2 = np.asarray(W_fc2, np.float32)
    b_fc2 = np.asarray(b_fc2, np.float32)

    apply_ln1 = not (np.all(ln1_w == 1.0) and np.all(ln1_b == 0.0))
    apply_ln2 = not (np.all(ln2_w == 1.0) and np.all(ln2_b == 0.0))
    apply_bv = bool(np.any(b_attn[2 * C :] != 0.0))
    apply_bqk = bool(np.any(b_attn[: 2 * C] != 0.0))

    nc = _build(apply_ln1, apply_ln2, apply_bv, apply_bqk)

    xf = x[0]  # [T, C]
    # Q/K psum partition permutation: psum partition p^ = 64j + 32e + q
    # holds pair-local feature 64e + 32j + q (e = head-in-pair, hd = 32j+q),
    # so each (head, j) chunk is a contiguous 32-partition block for the
    # DoubleRow repack DMAs.
    ph = np.arange(P)
    PERM = 64 * ((ph % 64) // 32) + 32 * (ph // 64) + ph % 32
    wqp = W_attn[:, :C].reshape(C, CT, P)[:, :, PERM].reshape(C, C)
    wq = wqp.reshape(CT, 2, HD, C)  # [kk, j, ki, f]
    wq8 = np.ascontiguousarray(wq.transpose(2, 0, 1, 3)).astype(F8)
    # DoubleRow packing: [ki, kk, j, f] = W[kk*128 + j*64 + ki, col0 + f]
    wkp = W_attn[:, C : 2 * C].reshape(C, CT, P)[:, :, PERM].reshape(C, C)
    wk = wkp.reshape(CT, 2, HD, C)  # [kk, j, ki, f]
    wk8 = np.ascontiguousarray(wk.transpose(2, 0, 1, 3)).astype(F8)
    wv = W_attn[:, 2 * C :].reshape(CT, 2, HD, C)
    wv8 = np.ascontiguousarray(wv.transpose(2, 0, 1, 3)).astype(F8)
    wproj_b = W_proj.astype(BF16)
    wfc_b = W_fc.astype(BF16)
    wfc2_b = W_fc2.astype(BF16)
    bqk = np.ascontiguousarray(b_attn[: 2 * C].reshape(2 * CT, P)[:, PERM].T)
    bproj = np.ascontiguousarray(b_proj.reshape(CT, P).T)
    bfc = np.ascontiguousarray(b_fc.reshape(HT, P).T)
    bfc2 = np.ascontiguousarray(b_fc2.reshape(CT, P).T)
    ones = np.ones((P, P), BF16)

    xtf = np.ascontiguousarray(xf.T.astype(BF16))
    in_maps = []
    qtok = []
    for c in range(NC):
        qt = _query_tokens(c)
        qtok.append(qt)
        xTq = np.ascontiguousarray(xf[qt, :].T)
        kk = np.arange(P)[:, None, None]
        dd = np.arange(8)[None, :, None]
        ii = np.arange(P)[None, None, :]
        masks = ((8 * ii + c - 128 * dd - kk) >= 0).astype(BF16)
        m = {
            "xtf": xtf, "xTq": xTq, "masks": masks, "ones": ones,
            "wk8": wk8, "wv8": wv8,
            "wq8": wq8, "wproj": wproj_b, "wfc": wfc_b, "wfc2": wfc2_b,
            "bqk": bqk, "bproj": bproj, "bfc": bfc, "bfc2": bfc2,
        }
        if apply_bv:
            m["bv"] = np.ascontiguousarray(np.broadcast_to(b_attn[2 * C :], (P, C)))
        if apply_ln1:
            m["ln1w"] = np.ascontiguousarray(ln1_w.reshape(CT, P).T)
            m["ln1b"] = np.ascontiguousarray(ln1_b.reshape(CT, P).T)
        if apply_ln2:
            m["ln2w"] = np.ascontiguousarray(ln2_w.reshape(CT, P).T)
            m["ln2b"] = np.ascontiguousarray(ln2_b.reshape(CT, P).T)
        in_maps.append(m)

    res = run_bass_kernel_spmd(nc, in_maps, list(range(NC)))

    out = np.empty((T, C), np.float32)
    for c in range(NC):
        out[qtok[c], :] = res.results[c]["outT"].T
    return out[None, :, :]


# revision 41
# speedup vs baseline: 1.2537x; 1.2537x over previous
"""Trainium2 Bass kernel for a GPT-2-style transformer block.

B=1, T=4096, C=768, H=12 heads (hd=64), causal attention, exact GELU MLP.

Distribution over 8 NeuronCores (single shared SPMD program; collectives on
this pool measure ~0.4-1 ms per call, so the design avoids them entirely):
  - Queries: mod-8 interleaved sharding (core c owns tokens t with t%8==c),
    which makes the causal-attention instruction structure IDENTICAL on all
    cores (one shared program; per-core behavior only via input data). The
    per-core diagonal-band causal masks are fed as inputs.
  - K/V: every core computes the full-sequence K^T/V locally (replicated
    matmul — far cheaper than any collective here). K and V projections run
    in fp8e4m3 with DoubleRow packing (2x PE rate) off a shared fp8 copy of
    the LN output (xp8).
  - K^T and Q^T are stored fp8, DoubleRow-packed along hd (head h on
    partitions 32*(h%3):+32 — matmul operands must base at 0/32/64 — group
    h//3, hd = 32*j + p with j a free dim), so the S matmuls also run at
    the 2x fp8 rate. The pack is 4 small SBUF DMAs per produced psum tile;
    the Q/K weight columns are host-permuted so psum partition 64j+32e+q
    holds pair-local feature 64e+32j+q, making each (head, j) chunk a
    contiguous 32-partition block.
  - V is SBUF-resident fp8 in natural [token, feature] layout with a
    prepended ones-column per head ([P, 32, 12, 65]): the AV matmul then
    accumulates the softmax row-sum into output partition 0 for free (no
    separate row-sum matmuls). Per-pair normalization: DVE reciprocal of
    the two row-sum rows, GPSIMD partition_broadcast, two muls.
  - Slab production (LN1 + K/V for token slabs 2..7) is INTERLEAVED into
    pair 0's attention at half-band granularity (band m half h needs
    exactly slab 2m+h), so the DVE/PE-heavy K/V work overlaps the
    Act-bound softmax exp of pair 0; pairs 1-5 then run Act-bound.
  - proj/LN2/MLP/residual: row-parallel on each core's own query rows.
    The MLP stays bf16: its output is ~30% of the residual, fp8 there
    costs ~1.6e-2 rel err (vs the ~1% attention branch where fp8 K/Q/V
    error is diluted ~100x).

LN statistics use ones-matmul partition reductions into a [P, 2, R] psum
tile SHARED (same pool tag) with the attention score tiles, keeping the
PSUM budget at 8 banks: scores/stats (2) + qk (2) + v (2) + y0/y1 (2).
rstd = Sqrt(reciprocal(var+eps)) (Rsqrt activation is banned; the Ln/Exp
trick thrashes the activation table at 1.28us per load since Ln and Exp
resolve to different tables). LN1 for the K/V slabs writes fp8 directly.
"""

import numpy as np
import ml_dtypes

import concourse.bacc as bacc
import concourse.mybir as mybir
import concourse.tile as tile
from concourse.bass_utils import run_bass_kernel_spmd

BF16 = ml_dtypes.bfloat16
F8 = ml_dtypes.float8_e4m3

# problem shape (hardcoded per harness contract)
T = 4096
C = 768
H = 12
HD = 64
EPS = 1e-5
NC = 8          # cores
R = 512         # tokens per core
P = 128
CT = C // P     # 6 feature tiles
QT = R // P     # 4 query tiles per core
KT = T // P     # 32 key tiles
PAIRS = H // 2  # 6 head pairs
HT = (4 * C) // P  # 24 hidden tiles

_CACHE = {}


def _ln_transposed(nc, tc, pool, pspool, xT, out_bf, ones_sb, w_col, b_col, apply_wb,
                   x_is_bf16=False, alt=0):
    """LayerNorm over the feature axis for [C, R]-transposed activations.

    xT: f32 (or bf16 with x_is_bf16) sbuf tile [P, CT, R]; out_bf: bf16 or
    fp8 tile. Stats via ones-matmul partition reduction (all-partition-
    broadcast results) into one [P, 2, R] psum tile (plane 0 = sum,
    plane 1 = sumsq); rstd = Sqrt(reciprocal(var+eps)).
    """
    f32 = mybir.dt.float32
    bf16 = mybir.dt.bfloat16
    if x_is_bf16:
        xb = xT
    else:
        xb = pool.tile([P, CT, R], bf16, tag="ln_xb")
    sq = pool.tile([P, CT, R], bf16, tag="ln_sq")
    for k in range(CT):
        eng = nc.gpsimd if (k + alt) % 3 == 2 else nc.vector
        if not x_is_bf16:
            eng.tensor_copy(xb[:, k, :], xT[:, k, :])
        eng.tensor_mul(sq[:, k, :], xb[:, k, :], xb[:, k, :])
    st = pspool.tile([P, 2, R], f32, tag="sl")
    for k in range(CT):
        nc.tensor.matmul(st[:, 0, :], ones_sb[:], xb[:, k, :], start=(k == 0), stop=(k == CT - 1))
    for k in range(CT):
        nc.tensor.matmul(st[:, 1, :], ones_sb[:], sq[:, k, :], start=(k == 0), stop=(k == CT - 1))
    # scratch tiles are reused in place: A = nmean -> -mu*rstd,
    # B = E[x^2] -> var -> rstd, tmp doubles as mean^2
    nmean = pool.tile([P, R], f32, tag="ln_a")
    rstd = pool.tile([P, R], f32, tag="ln_b")
    tmp = pool.tile([P, R], f32, tag="ln_tmp")
    tmp2 = pool.tile([P, R], f32, tag="ln_tmp2")
    sm = nc.vector if alt == 0 else nc.gpsimd
    AFT = mybir.ActivationFunctionType
    # stats scalar chain rides the (idle) scalar engine: nmean, E[x^2], mean^2
    nc.scalar.activation(nmean[:], st[:, 0, :], AFT.Copy, scale=-1.0 / C)
    nc.scalar.activation(rstd[:], st[:, 1, :], AFT.Copy, scale=1.0 / C)  # E[x^2]
    nc.scalar.activation(tmp[:], nmean[:], AFT.Square)                   # mean^2
    # var = (E[x^2] + eps) - mean^2
    sm.scalar_tensor_tensor(
        rstd[:], rstd[:], EPS, tmp[:], mybir.AluOpType.add, mybir.AluOpType.subtract
    )
    nc.vector.reciprocal(rstd[:], rstd[:])
    nc.scalar.activation(rstd[:], rstd[:], AFT.Sqrt)
    nmr = nmean
    sm.tensor_mul(nmr[:], nmean[:], rstd[:])           # -mu*rstd
    for k in range(CT):
        eng = nc.gpsimd if (k + alt) % 3 == 2 else nc.vector
        t = tmp2 if (k + alt) % 3 == 2 else tmp
        eng.tensor_mul(t[:], xT[:, k, :], rstd[:])
        if apply_wb:
            eng.tensor_add(t[:], t[:], nmr[:])
            eng.tensor_scalar(
                out_bf[:, k, :], t[:], w_col[:, k : k + 1], b_col[:, k : k + 1],
                mybir.AluOpType.mult, mybir.AluOpType.add,
            )
        else:
            eng.tensor_add(out_bf[:, k, :], t[:], nmr[:])


def _build(apply_ln1, apply_ln2, apply_bv, apply_bqk=False, sim_no_cc=False, reps=1):
    key = (apply_ln1, apply_ln2, apply_bv, apply_bqk, reps)
    if key in _CACHE:
        return _CACHE[key]

    f32 = mybir.dt.float32
    bf16 = mybir.dt.bfloat16
    f8 = mybir.dt.float8e4
    AF = mybir.ActivationFunctionType
    DR = mybir.MatmulPerfMode.DoubleRow

    nc = bacc.Bacc("TRN2", target_bir_lowering=False, debug=False, num_devices=NC)

    xtf_d = nc.declare_dram_parameter("xtf", [C, T], bf16, isOutput=False)
    xTq_d = nc.declare_dram_parameter("xTq", [C, R], f32, isOutput=False)
    masks_d = nc.declare_dram_parameter("masks", [P, 8, P], bf16, isOutput=False)
    ones_d = nc.declare_dram_parameter("ones", [P, P], bf16, isOutput=False)
    wq8_d = nc.declare_dram_parameter("wq8", [HD, CT, 2, C], f8, isOutput=False)
    wk8_d = nc.declare_dram_parameter("wk8", [HD, CT, 2, C], f8, isOutput=False)
    wv8_d = nc.declare_dram_parameter("wv8", [HD, CT, 2, C], f8, isOutput=False)
    wproj_d = nc.declare_dram_parameter("wproj", [C, C], bf16, isOutput=False)
    wfc_d = nc.declare_dram_parameter("wfc", [C, 4 * C], bf16, isOutput=False)
    wfc2_d = nc.declare_dram_parameter("wfc2", [4 * C, C], bf16, isOutput=False)
    bqk_d = nc.declare_dram_parameter("bqk", [P, 2 * CT], f32, isOutput=False)
    bproj_d = nc.declare_dram_parameter("bproj", [P, CT], f32, isOutput=False)
    bfc_d = nc.declare_dram_parameter("bfc", [P, HT], f32, isOutput=False)
    bfc2_d = nc.declare_dram_parameter("bfc2", [P, CT], f32, isOutput=False)
    if apply_bv:
        bv_d = nc.declare_dram_parameter("bv", [P, C], f32, isOutput=False)
    if apply_ln1:
        ln1w_d = nc.declare_dram_parameter("ln1w", [P, CT], f32, isOutput=False)
        ln1b_d = nc.declare_dram_parameter("ln1b", [P, CT], f32, isOutput=False)
    if apply_ln2:
        ln2w_d = nc.declare_dram_parameter("ln2w", [P, CT], f32, isOutput=False)
        ln2b_d = nc.declare_dram_parameter("ln2b", [P, CT], f32, isOutput=False)
    outT_d = nc.declare_dram_parameter("outT", [C, R], f32, isOutput=True)

    with tile.TileContext(nc) as tc:
        with (
            tc.tile_pool(name="const", bufs=1) as const,
            tc.tile_pool(name="mid", bufs=1) as mid,
        ):
            ones_sb = const.tile([P, P], bf16)
            nc.sync.dma_start(ones_sb[:], ones_d[:])
            bqk_sb = const.tile([P, 2 * CT], f32)
            nc.sync.dma_start(bqk_sb[:], bqk_d[:])
            bproj_sb = const.tile([P, CT], f32)
            nc.gpsimd.dma_start(bproj_sb[:], bproj_d[:])
            bfc_sb = const.tile([P, HT], f32)
            nc.gpsimd.dma_start(bfc_sb[:], bfc_d[:])
            bfc2_sb = const.tile([P, CT], f32)
            nc.gpsimd.dma_start(bfc2_sb[:], bfc2_d[:])
            if apply_bv:
                bv_sb = const.tile([P, C], f32)
                nc.sync.dma_start(bv_sb[:], bv_d[:])
            ln1w_sb = ln1b_sb = ln2w_sb = ln2b_sb = None
            if apply_ln1:
                ln1w_sb = const.tile([P, CT], f32)
                ln1b_sb = const.tile([P, CT], f32)
                nc.sync.dma_start(ln1w_sb[:], ln1w_d[:])
                nc.sync.dma_start(ln1b_sb[:], ln1b_d[:])
            if apply_ln2:
                ln2w_sb = const.tile([P, CT], f32)
                ln2b_sb = const.tile([P, CT], f32)
                nc.sync.dma_start(ln2w_sb[:], ln2w_d[:])
                nc.sync.dma_start(ln2b_sb[:], ln2b_d[:])
            xTq_sb = const.tile([P, CT, R], f32)
            nc.sync.dma_start(xTq_sb[:], xTq_d.rearrange("(o p) t -> p o t", p=P))

            # mid-lifetime tiles
            q8 = mid.tile([P, H // 3, 2, R], f8)
            ynorm_sb = mid.tile([P, CT, R], bf16)  # normalized attn out (y^T)

            for _rep in range(reps):
                # pools/psum live across the fused produce+attend phase:
                # scores/LN-stats share one pool tag (2 banks) + qk(2) +
                # v(2) + y0/y1(2) = 8 banks.
                ktp_cm = tc.tile_pool(name="ktp", bufs=1)
                ktp = ktp_cm.__enter__()
                kt8 = ktp.tile([P, H // 3, 2, T], f8)
                v_res = ktp.tile([P, KT, H, 65], f8)
                masks_sb = ktp.tile([P, 8, P], bf16)
                nc.gpsimd.dma_start(masks_sb[:], masks_d[:])
                bp_cm = tc.tile_pool(name="bp", bufs=2)
                bp = bp_cm.__enter__()
                bn_cm = tc.tile_pool(name="bn", bufs=1)
                bn = bn_cm.__enter__()
                ps_qk_cm = tc.tile_pool(name="ps_qk", bufs=2, space="PSUM")
                ps_qk = ps_qk_cm.__enter__()
                ps_v_cm = tc.tile_pool(name="ps_v", bufs=2, space="PSUM")
                ps_v = ps_v_cm.__enter__()
                ps_sl_cm = tc.tile_pool(name="ps_sl", bufs=2, space="PSUM")
                ps_sl = ps_sl_cm.__enter__()
                lnp_cm = tc.tile_pool(name="lnp", bufs=1)
                lnp = lnp_cm.__enter__()
                slabp_cm = tc.tile_pool(name="slabp", bufs=2)
                slabp = slabp_cm.__enter__()

                # ones column of v_res (read by every AV matmul)
                nc.gpsimd.tensor_copy(
                    v_res[:, :, :, 0:1],
                    ones_sb[:, 0:1].to_broadcast((P, KT, H, 1)),
                )

                # ---- setup: Q^T (fp8 DoubleRow-packed) ----
                with tc.tile_pool(name="qpool", bufs=1) as qpool:
                    wq8_sb = qpool.tile([HD, CT, 2, C], f8)
                    nc.sync.dma_start(wq8_sb[:], wq8_d[:])
                    xlnq8 = qpool.tile([P, CT, R], f8)
                    _ln_transposed(nc, tc, lnp, ps_sl, xTq_sb, xlnq8, ones_sb,
                                   ln1w_sb, ln1b_sb, apply_ln1)
                    xqp8 = qpool.tile([HD, CT, 2, R], f8)
                    nc.scalar.dma_start(xqp8[:, :, 0, :], xlnq8[0:HD, :, :])
                    nc.scalar.dma_start(xqp8[:, :, 1, :], xlnq8[HD:P, :, :])
                    for f in range(CT):
                        ps = ps_qk.tile([P, R], f32, tag="qk_ps")
                        for k in range(CT):
                            nc.tensor.matmul(
                                ps[:], wq8_sb[:, k, :, P * f : P * (f + 1)],
                                xqp8[:, k, :, :], start=(k == 0), stop=(k == CT - 1),
                                perf_mode=DR,
                            )
                        qst = qpool.tile([P, R], f8, tag="qst", name="qst", bufs=1)
                        nc.vector.tensor_scalar(
                            qst[:], ps[:], bqk_sb[:, f : f + 1], None,
                            mybir.AluOpType.add,
                        )
                        for e in range(2):
                            h = 2 * f + e
                            for j in range(2):
                                nc.scalar.dma_start(
                                    q8[32 * (h % 3) : 32 * (h % 3) + 32, h // 3, j, :],
                                    qst[64 * j + 32 * e : 64 * j + 32 * e + 32, :],
                                )

                    # slab tiles (shared with the interleaved phase below)
                    wk8_sb = slabp.tile([HD, CT, 2, C], f8, bufs=1)
                    nc.gpsimd.dma_start(wk8_sb[:], wk8_d[:])
                    wv8_sb = slabp.tile([HD, CT, 2, C], f8, bufs=1)
                    nc.gpsimd.dma_start(wv8_sb[:], wv8_d[:])
                    xtf_r = xtf_d.rearrange("(o p) t -> p o t", p=P)

                    def emit_slab(s):
                        """LN1 + K^T + V for token slab s (fp8 DoubleRow)."""
                        xv = slabp.tile([P, CT, R], bf16, tag="xv", name="xv")
                        xeng = nc.sync if s % 2 == 0 else nc.gpsimd
                        xeng.dma_start(xv[:], xtf_r[:, :, R * s : R * (s + 1)])
                        xln8 = slabp.tile([P, CT, R], f8, tag="xln8", name="xln8")
                        _ln_transposed(nc, tc, lnp, ps_sl, xv, xln8, ones_sb,
                                       ln1w_sb, ln1b_sb, apply_ln1, x_is_bf16=True)
                        xp8 = slabp.tile([HD, CT, 2, R], f8, tag="xp8", name="xp8")
                        nc.scalar.dma_start(xp8[:, :, 0, :], xln8[0:HD, :, :])
                        nc.scalar.dma_start(xp8[:, :, 1, :], xln8[HD:P, :, :])

                        for f in range(CT):
                            ps = ps_qk.tile([P, R], f32, tag="qk_ps")
                            for k in range(CT):
                                nc.tensor.matmul(
                                    ps[:], wk8_sb[:, k, :, P * f : P * (f + 1)],
                                    xp8[:, k, :, :], start=(k == 0), stop=(k == CT - 1),
                                    perf_mode=DR,
                                )
                            kst = slabp.tile([P, R], f8, tag="kst", name="kst")
                            keng = nc.vector if (f + s) % 2 == 0 else nc.gpsimd
                            if apply_bqk:
                                keng.tensor_scalar(
                                    kst[:], ps[:],
                                    bqk_sb[:, CT + f : CT + f + 1], None,
                                    mybir.AluOpType.add,
                                )
                            else:
                                keng.tensor_copy(kst[:], ps[:])
                            rot = [nc.sync, nc.scalar, nc.gpsimd, nc.sync]
                            for e in range(2):
                                h = 2 * f + e
                                for j in range(2):
                                    rot[(2 * e + j + f) % 4].dma_start(
                                        kt8[32 * (h % 3) : 32 * (h % 3) + 32, h // 3, j,
                                            R * s : R * (s + 1)],
                                        kst[64 * j + 32 * e : 64 * j + 32 * e + 32, :],
                                    )

                        for t in range(QT):
                            kt = QT * s + t
                            for hh in range(2):
                                ps = ps_v.tile([P, 384], f32, tag="v_ps")
                                for k in range(CT):
                                    nc.tensor.matmul(
                                        ps[:], xp8[:, k, :, P * t : P * (t + 1)],
                                        wv8_sb[:, k, :, 384 * hh : 384 * (hh + 1)],
                                        start=(k == 0), stop=(k == CT - 1),
                                        perf_mode=DR,
                                    )
                                dst = v_res[:, kt, 6 * hh : 6 * (hh + 1), 1:65]
                                psv = ps[:].rearrange("p (h f) -> p h f", h=6)
                                veng = nc.vector if (t + hh) % 2 == 0 else nc.gpsimd
                                if apply_bv:
                                    bvv = bv_sb[:, 384 * hh : 384 * (hh + 1)].rearrange(
                                        "p (h f) -> p h f", h=6
                                    )
                                    veng.tensor_add(dst, psv, bvv)
                                else:
                                    veng.tensor_copy(dst, psv)

                    for s in range(NC):
                        emit_slab(s)

                slabp_cm.__exit__(None, None, None)
                lnp_cm.__exit__(None, None, None)
                ps_sl_cm.__exit__(None, None, None)
                ps_v_cm.__exit__(None, None, None)
                ps_qk_cm.__exit__(None, None, None)
                ps_s_cm = tc.tile_pool(name="ps_s", bufs=3, space="PSUM")
                ps_s = ps_s_cm.__enter__()
                ps_y_cm = tc.tile_pool(name="ps_y", bufs=1, space="PSUM")
                ps_y = ps_y_cm.__enter__()

                # ---------------- attention ----------------
                if True:
                    for pr in range(PAIRS):
                        y0_ps = ps_y.tile([P, R], f32, tag="y0")
                        y1_ps = ps_y.tile([P, R], f32, tag="y1")
                        for m in range(4):  # bands of 8 key-tiles
                            N = P * (4 - m)
                            # p_band is h-major: [P, head, ktile-in-band, R]
                            p_band = bp.tile([P, 2, 8, R], bf16, tag="p")
                            G = (1, 1, 2, 4)[m]
                            for half in range(2):
                                ng = 4 // G  # exp groups per half
                                for g in range(half * ng, (half + 1) * ng):
                                    s_ps = ps_s.tile([P, 2, R], f32, tag="sl", name="s_ps")
                                    for dg in range(G):
                                        d = g * G + dg
                                        k = 8 * m + d
                                        for e in range(2):
                                            h = 2 * pr + e
                                            hb = 32 * (h % 3)
                                            hg = h // 3
                                            nc.tensor.matmul(
                                                s_ps[:, e, dg * N : (dg + 1) * N],
                                                kt8[hb : hb + 32, hg, :, P * k : P * (k + 1)],
                                                q8[hb : hb + 32, hg, :, 0:N],
                                                perf_mode=DR,
                                                skip_group_check=True,
                                            )
                                    if G == 1:
                                        nc.scalar.activation(
                                            p_band[:, :, g, :N], s_ps[:, :, :N],
                                            AF.Exp, scale=0.125,
                                        )
                                    else:
                                        nc.scalar.activation(
                                            p_band[:, :, g * G : (g + 1) * G, :N],
                                            s_ps[:].rearrange("p h (a n) -> p h a n", n=N),
                                            AF.Exp, scale=0.125,
                                        )
                                for d in range(4 * half, 4 * half + 4):
                                    k = 8 * m + d
                                    # causal mask on the diagonal col-group
                                    meng = nc.gpsimd if d % 3 == 1 else nc.vector
                                    meng.tensor_mul(
                                        p_band[:, :, d, N - P : N],
                                        p_band[:, :, d, N - P : N],
                                        masks_sb[:, d : d + 1, :].to_broadcast((P, 2, P)),
                                    )
                                    # AV + row-sum in one matmul per head (V
                                    # ones-column -> row-sum in partition 0)
                                    nc.tensor.matmul(
                                        y0_ps[0:65, 0:N], v_res[:, k, 2 * pr, :],
                                        p_band[:, 0, d, :N],
                                        start=(k == 0), stop=(k == KT - 1),
                                        skip_group_check=True,
                                    )
                                    nc.tensor.matmul(
                                        y1_ps[0:65, 0:N], v_res[:, k, 2 * pr + 1, :],
                                        p_band[:, 1, d, :N],
                                        start=(k == 0), stop=(k == KT - 1),
                                        skip_group_check=True,
                                    )
                        # normalization: recip the row-sum rows, partition-
                        # broadcast on GPSIMD, scale the AV rows
                        recip0 = bn.tile([1, R], bf16, tag="recip0")
                        recip1 = bn.tile([1, R], bf16, tag="recip1")
                        with nc.allow_low_precision(reason="softmax denom recip"):
                            nc.vector.reciprocal(recip0[:], y0_ps[0:1, :])
                            nc.vector.reciprocal(recip1[:], y1_ps[0:1, :])
                        bc0 = bn.tile([P, R], bf16, tag="bc0")
                        bc1 = bn.tile([P, R], bf16, tag="bc1")
                        nc.gpsimd.partition_broadcast(bc0[:], recip0[:])
                        nc.gpsimd.partition_broadcast(bc1[:], recip1[:])
                        nc.vector.tensor_mul(
                            ynorm_sb[0:HD, pr, :], y0_ps[1:65, :], bc0[0:HD, :]
                        )
                        nc.vector.tensor_mul(
                            ynorm_sb[HD:P, pr, :], y1_ps[1:65, :], bc1[HD:P, :]
                        )

                ps_y_cm.__exit__(None, None, None)
                ps_s_cm.__exit__(None, None, None)
                bn_cm.__exit__(None, None, None)
                bp_cm.__exit__(None, None, None)
                ktp_cm.__exit__(None, None, None)

                # ---------------- Phase C: proj + LN2 + MLP + out ----------------
                with (
                    tc.tile_pool(name="mlpp", bufs=1) as mlpp,
                    tc.tile_pool(name="lnp2", bufs=1) as lnp2,
                ):
                    # weight loads split across queues so no single 14us DMA
                    # gates the MLP
                    wproj_sb = mlpp.tile([P, CT, C], bf16)
                    wproj_r = wproj_d.rearrange("(o p) f -> p o f", p=P)
                    nc.sync.dma_start(wproj_sb[:, 0:3, :], wproj_r[:, 0:3, :])
                    nc.gpsimd.dma_start(wproj_sb[:, 3:6, :], wproj_r[:, 3:6, :])
                    wfc_sb = mlpp.tile([P, CT, 4 * C], bf16)
                    wfc_r = wfc_d.rearrange("(o p) f -> p o f", p=P)
                    dengs = [nc.sync, nc.gpsimd, nc.scalar, nc.gpsimd]
                    for ch in range(4):
                        dengs[ch % 4].dma_start(
                            wfc_sb[:, :, C * ch : C * (ch + 1)],
                            wfc_r[:, :, C * ch : C * (ch + 1)],
                        )
                    z_sb = mlpp.tile([P, CT, R], f32)      # residual stream x+attn
                    xln2_sb = mlpp.tile([P, CT, R], bf16)
                    wfc2_sb = mlpp.tile([P, HT, C], bf16)
                    wfc2_r = wfc2_d.rearrange("(o p) f -> p o f", p=P)
                    for ch in range(4):
                        dengs[ch % 4].dma_start(
                            wfc2_sb[:, CT * ch : CT * (ch + 1), :],
                            wfc2_r[:, CT * ch : CT * (ch + 1), :],
                        )
                    with (
                        tc.tile_pool(name="ps_proj", bufs=2, space="PSUM") as ps_proj,
                        tc.tile_pool(name="ps_ln2", bufs=1, space="PSUM") as ps_ln2,
                    ):
                        for f in range(CT):
                            ps = ps_proj.tile([P, R], f32, tag="proj")
                            for k in range(CT):
                                nc.tensor.matmul(
                                    ps[:], wproj_sb[:, k, P * f : P * (f + 1)],
                                    ynorm_sb[:, k, :], start=(k == 0), stop=(k == CT - 1),
                                )
                            # z = (proj + b_proj) + x
                            nc.vector.scalar_tensor_tensor(
                                z_sb[:, f, :], ps[:], bproj_sb[:, f : f + 1], xTq_sb[:, f, :],
                                mybir.AluOpType.add, mybir.AluOpType.add,
                            )
                        _ln_transposed(nc, tc, lnp2, ps_ln2, z_sb, xln2_sb, ones_sb,
                                       ln2w_sb, ln2b_sb, apply_ln2)

                    h_sb = mlpp.tile([P, CT, R], bf16)
                    with (
                        tc.tile_pool(name="ps_fc1", bufs=2, space="PSUM") as ps_fc1,
                        tc.tile_pool(name="ps_o", bufs=1, space="PSUM") as ps_o,
                    ):
                        o_ps = [ps_o.tile([P, R], f32, tag=f"o{f}", name=f"o_ps{f}") for f in range(CT)]
                        for chunk in range(4):
                            for hf in range(CT):
                                hh = CT * chunk + hf
                                ps = ps_fc1.tile([P, R], f32, tag="fc1")
                                for k in range(CT):
                                    nc.tensor.matmul(
                                        ps[:], wfc_sb[:, k, P * hh : P * (hh + 1)],
                                        xln2_sb[:, k, :], start=(k == 0), stop=(k == CT - 1),
                                    )
                                nc.scalar.activation(
                                    h_sb[:, hf, :], ps[:], AF.Gelu, bias=bfc_sb[:, hh : hh + 1]
                                )
                                for f in range(CT):
                                    nc.tensor.matmul(
                                        o_ps[f][:], wfc2_sb[:, hh, P * f : P * (f + 1)],
                                        h_sb[:, hf, :], start=(hh == 0), stop=(hh == HT - 1),
                                    )
                        for f in range(CT):
                            # out = (o + b_fc2) + z, in place over z
                            nc.vector.scalar_tensor_tensor(
                                z_sb[:, f, :], o_ps[f][:], bfc2_sb[:, f : f + 1], z_sb[:, f, :],
                                mybir.AluOpType.add, mybir.AluOpType.add,
                            )
                            nc.sync.dma_start(outT_d[P * f : P * (f + 1), :], z_sb[:, f, :])

    nc.compile()
    _CACHE[key] = nc
    return nc


def _query_tokens(c):
    """Token ids owned by core c, in on-chip column order (j desc, i asc)."""
    return np.concatenate([1024 * j + 8 * np.arange(P) + c for j in (3, 2, 1, 0)])


def kernel(x, ln1_w, ln1_b, W_attn, b_attn, W_proj, b_proj,
           ln2_w, ln2_b, W_fc, b_fc, W_fc2, b_fc2):
    x = np.asarray(x, np.float32)
    ln1_w = np.asarray(ln1_w, np.float32)
    ln1_b = np.asarray(ln1_b, np.float32)
    W_attn = np.asarray(W_attn, np.float32)
    b_attn = np.asarray(b_attn, np.float32)
    W_proj = np.asarray(W_proj, np.float32)
    b_proj = np.asarray(b_proj, np.float32)
    ln2_w = np.asarray(ln2_w, np.float32)
    ln2_b = np.asarray(ln2_b, np.float32)
    W_fc = np.asarray(W_fc, np.float32)
    b_fc = np.asarray(b_fc, np.float32)
    W_fc2 = np.asarray(W_fc2, np.float32)
    b_fc2 = np.asarray(b_fc2, np.float32)

    apply_ln1 = not (np.all(ln1_w == 1.0) and np.all(ln1_b == 0.0))
    apply_ln2 = not (np.all(ln2_w == 1.0) and np.all(ln2_b == 0.0))
    apply_bv = bool(np.any(b_attn[2 * C :] != 0.0))
    apply_bqk = bool(np.any(b_attn[: 2 * C] != 0.0))

    nc = _build(apply_ln1, apply_ln2, apply_bv, apply_bqk)

    xf = x[0]  # [T, C]
    # Q/K psum partition permutation: psum partition p^ = 64j + 32e + q
    # holds pair-local feature 64e + 32j + q (e = head-in-pair, hd = 32j+q),
    # so each (head, j) chunk is a contiguous 32-partition block for the
    # DoubleRow repack DMAs.
    ph = np.arange(P)
    PERM = 64 * ((ph % 64) // 32) + 32 * (ph // 64) + ph % 32
    wqp = W_attn[:, :C].reshape(C, CT, P)[:, :, PERM].reshape(C, C)
    wq = wqp.reshape(CT, 2, HD, C)  # [kk, j, ki, f]
    wq8 = np.ascontiguousarray(wq.transpose(2, 0, 1, 3)).astype(F8)
    # DoubleRow packing: [ki, kk, j, f] = W[kk*128 + j*64 + ki, col0 + f]
    wkp = W_attn[:, C : 2 * C].reshape(C, CT, P)[:, :, PERM].reshape(C, C)
    wk = wkp.reshape(CT, 2, HD, C)  # [kk, j, ki, f]
    wk8 = np.ascontiguousarray(wk.transpose(2, 0, 1, 3)).astype(F8)
    wv = W_attn[:, 2 * C :].reshape(CT, 2, HD, C)
    wv8 = np.ascontiguousarray(wv.transpose(2, 0, 1, 3)).astype(F8)
    wproj_b = W_proj.astype(BF16)
    wfc_b = W_fc.astype(BF16)
    wfc2_b = W_fc2.astype(BF16)
    bqk = np.ascontiguousarray(b_attn[: 2 * C].reshape(2 * CT, P)[:, PERM].T)
    bproj = np.ascontiguousarray(b_proj.reshape(CT, P).T)
    bfc = np.ascontiguousarray(b_fc.reshape(HT, P).T)
    bfc2 = np.ascontiguousarray(b_fc2.reshape(CT, P).T)
    ones = np.ones((P, P), BF16)

    xtf = np.ascontiguousarray(xf.T.astype(BF16))
    in_maps = []
    qtok = []
    for c in range(NC):
        qt = _query_tokens(c)
        qtok.append(qt)
        xTq = np.ascontiguousarray(xf[qt, :].T)
        kk = np.arange(P)[:, None, None]
        dd = np.arange(8)[None, :, None]
        ii = np.arange(P)[None, None, :]
        masks = ((8 * ii + c - 128 * dd - kk) >= 0).astype(BF16)
        m = {
            "xtf": xtf, "xTq": xTq, "masks": masks, "ones": ones,
            "wk8": wk8, "wv8": wv8,
            "wq8": wq8, "wproj": wproj_b, "wfc": wfc_b, "wfc2": wfc2_b,
            "bqk": bqk, "bproj": bproj, "bfc": bfc, "bfc2": bfc2,
        }
        if apply_bv:
            m["bv"] = np.ascontiguousarray(np.broadcast_to(b_attn[2 * C :], (P, C)))
        if apply_ln1:
            m["ln1w"] = np.ascontiguousarray(ln1_w.reshape(CT, P).T)
            m["ln1b"] = np.ascontiguousarray(ln1_b.reshape(CT, P).T)
        if apply_ln2:
            m["ln2w"] = np.ascontiguousarray(ln2_w.reshape(CT, P).T)
            m["ln2b"] = np.ascontiguousarray(ln2_b.reshape(CT, P).T)
        in_maps.append(m)

    res = run_bass_kernel_spmd(nc, in_maps, list(range(NC)))

    out = np.empty((T, C), np.float32)
    for c in range(NC):
        out[qtok[c], :] = res.results[c]["outT"].T
    return out[None, :, :]
